# revision 19
# baseline (speedup 1.0000x reference)
"""Trainium2 Bass kernel for nn_Attention_32091995636193.

Dense transformer attention block:
  qkv = x @ qkv_w.T ; per-head LN(q), LN(k) over head_dim ; k centered over
  seq ; softmax(q*scale @ k^T) @ v ; out @ proj_w.T + proj_b.

Sharding over 8 NeuronCores: data parallel on batch (B=2) x tensor parallel
on heads (16 heads -> 4 per core). Core c handles batch c//4, heads
4*(c%4) .. 4*(c%4)+3. Each core computes its partial projection output
[N, C]; the host sums the 4 partials per batch and adds proj_b.

Per-core device program:
  1. qkv matmuls kc-OUTER over t-groups of 3 so PE starts as soon as the
     first xt k-tile DMA lands (combined [wk|wq|wv] weight, 2 matmuls/t/kc)
  2. LN stats batched per group on bf16 SBUF copies (DVE 2x modes);
     broadcast-AP apply; PE-transpose in t-batches of 4
  3. attention chunks in nh-MAJOR order. ALL attention matmuls run
     full-array (scores use zero-padded per-head kTz so K=128; v5 is
     padded to M=128): half-array matmuls keep the PE HAM clock-gate
     throttled at ~1.8GHz, full-array runs at 2.4GHz. exp mostly on ACT
     with a 4/16 Schraudolph (int16-bitcast bf16) split on DVE;
     denominator via ones-row in the attnv matmul
  4. deferred normalization: reciprocal via 128-partition DMA reshape
     (ACT ln/exp chain for the last head to skip the DMA latency),
     K=1 ones-matmul broadcast, DVE multiply - interleaved into the
     attention chunk stream per head as denominators become ready
  5. projection + output DMA interleaved into chunks 6-7; only the
     nh=1 second half of proj remains as tail
"""

import os
import sys

for _p in ("/opt/trn_rl_repo",):
    if _p not in sys.path and os.path.isdir(_p):
        sys.path.append(_p)

import numpy as np

# ---------------------------------------------------------------------------
# BIR legalizer: the pinned walrus build supports at most ONE sync wait per
# instruction, but Tile's scheduler attaches several. Split extra waits onto
# NoOp instructions inserted immediately before (same engine => same NX
# order => identical semantics).
# ---------------------------------------------------------------------------
import orjson


def _legalize_bir_json_bytes(raw: bytes) -> bytes:
    j = orjson.loads(raw)
    counter = 0
    for f in j.get("functions", []):
        for blk in f.get("blocks", []):
            insts = blk.get("instructions")
            if not insts:
                continue
            out = []
            for ins in insts:
                si = ins.get("sync_info")
                waits = si.get("on_wait") if si else None
                if waits and len(waits) > 1:
                    engine = ins.get("engine")
                    for w in waits[:-1]:
                        counter += 1
                        nop = {
                            "name": f"WSPLIT-{counter}",
                            "opcode": "NoOp",
                            "engine": engine,
                            "ins": [],
                            "outs": [],
                            "sync_info": {"on_wait": [w], "on_update": []},
                        }
                        if "debug" in ins:
                            nop["debug"] = ins["debug"]
                        out.append(nop)
                    si["on_wait"] = [waits[-1]]
                out.append(ins)
            blk["instructions"] = out
    return orjson.dumps(j)


_PATCHED = False


def _install_patch():
    global _PATCHED
    if _PATCHED:
        return
    import concourse.bass as bass

    orig = bass.Bass.to_json_bytes

    def patched(self):
        return _legalize_bir_json_bytes(orig(self))

    bass.Bass.to_json_bytes = patched
    _PATCHED = True


# ---------------------------------------------------------------------------
# Problem constants (hardcoded per the harness contract)
# ---------------------------------------------------------------------------
B = 2
N = 2048
C = 1024
H = 16
D = 64
SCALE = D ** -0.5
EPS = 1e-5
NCORES = 8
HPC = H // 4          # heads per core = 4
DPC = HPC * D         # channels per core = 256
NT = N // 128         # 16 n-tiles
KT = C // 128         # 8 contraction tiles

# Schraudolph exp approximation in bf16 (int16 bitcast):
#   exp(x) ~ bitcast_bf16(int16(round(x * 128/ln2 + (127 - c)*128)))
SCH_A = 128.0 / np.log(2.0)          # 184.6650
SCH_B = (127.0 - 0.04367) * 128.0    # 16250.41
# mt indices whose exp runs on DVE instead of ACT (per chunk)
DVE_MTS = frozenset((1, 4, 7, 10, 13))

_nc_cache = {}


def _build_program(ln_trivial: bool):
    import concourse.bass as bass
    import concourse.mybir as mybir
    import concourse.tile as tile

    fr = mybir.dt.float32r
    f32 = mybir.dt.float32
    bf = mybir.dt.bfloat16
    i16 = mybir.dt.int16
    AX = mybir.AxisListType
    OP = mybir.AluOpType
    ACTF = mybir.ActivationFunctionType

    nc = bass.Bass()
    xt = nc.declare_dram_parameter("xt", [C, N], bf, isOutput=False)
    # combined qkv weight slice, column order [wk | wq | wv] (DPC each)
    wqkv = nc.declare_dram_parameter("wqkv", [C, 3 * DPC], bf, isOutput=False)
    wp = nc.declare_dram_parameter("wp", [DPC, C], bf, isOutput=False)
    ones64 = nc.declare_dram_parameter("ones64", [1, D], fr, isOutput=False)
    ident_in = nc.declare_dram_parameter("ident", [128, 128], bf, isOutput=False)
    if not ln_trivial:
        gqb = nc.declare_dram_parameter("gqb", [128, 4, D], f32, isOutput=False)
        bqb = nc.declare_dram_parameter("bqb", [128, 4, D], f32, isOutput=False)
        gkb = nc.declare_dram_parameter("gkb", [128, 4, D], f32, isOutput=False)
        bkb = nc.declare_dram_parameter("bkb", [128, 4, D], f32, isOutput=False)
    out = nc.declare_dram_parameter("out", [N, C], f32, isOutput=True)

    with tile.TileContext(nc) as tc:
        with tc.tile_pool(name="const", bufs=1) as cpool, \
             tc.tile_pool(name="persist", bufs=1) as bpool:

            ident = cpool.tile([128, 128], bf)
            ones_t = cpool.tile([1, D], fr)
            wsrc0 = cpool.tile([128, 512], bf)
            nc.vector.memset(wsrc0[:], 0.01)
            eps_t = cpool.tile([128, 1], f32)
            nc.vector.memset(eps_t[:], EPS)
            eps64_t = cpool.tile([128, 1], f32)
            nc.vector.memset(eps64_t[:], D * EPS)

            # ---- persistent tensors (live into attention/proj) --------
            wp_s = bpool.tile([128, 2, C], bf, name="wp_s")
            v5 = bpool.tile([128, NT * HPC, 128], bf, name="v5")
            nc.gpsimd.memset(v5[:, :, 64:65], 1.0)
            nc.gpsimd.memset(v5[:, :, 65:128], 0.0)
            qT = [bpool.tile([128, N], bf, name=f"qT{p}") for p in range(2)]
            kTz = [bpool.tile([128, N], bf, name=f"kTz{h}") for h in range(4)]
            for h in range(4):
                off = 64 * (h % 2)
                nc.vector.memset(kTz[h][64 - off:128 - off, :], 0.0)
            outT = [bpool.tile([128, N], bf, name=f"outT{p}") for p in range(2)]

            with tc.tile_pool(name="ph13", bufs=1) as wpool, \
                 tc.tile_pool(name="qkv_ps", bufs=1, space="PSUM") as qps, \
                 tc.tile_pool(name="sq_pool", bufs=2) as sqpool, \
                 tc.tile_pool(name="stat_tmp", bufs=2) as stp, \
                 tc.tile_pool(name="tp_ps", bufs=2, space="PSUM") as tps:
                # ---- load inputs/weights for phases 1-3 ---------------
                # per-k-tile DMAs, interleaved weight/xt so the kc-outer
                # accumulation can start as soon as k-tile 0 lands
                wqkv_s = wpool.tile([128, KT, 3 * DPC], bf, name="wqkv_s")
                wqkv_r = wqkv.rearrange("(k p) d -> p k d", p=128)
                xt_s = wpool.tile([128, KT, N], bf, name="xt_s")
                xt_r = xt.rearrange("(k p) n -> p k n", p=128)
                for kc in range(KT):
                    eng = (nc.sync, nc.scalar, nc.gpsimd)[kc % 3]
                    eng.dma_start(xt_s[:, kc:kc + 1], xt_r[:, kc:kc + 1])
                    eng.dma_start(wqkv_s[:, kc:kc + 1], wqkv_r[:, kc:kc + 1])
                # late-needed loads at the tail of the queues
                nc.sync.dma_start(ident[:], ident_in[:])
                nc.scalar.dma_start(ones_t[:], ones64[:])
                nc.gpsimd.dma_start(wp_s[:], wp.rearrange("(k p) n -> p k n", p=128))
                # warm the PE clock from a memset scratch: starts right at
                # program begin, no DMA dependency
                # (borrows the pg0 psum slot; W-after-W serializes safely)
                wut = qps.tile([128, 3 * DPC], f32, tag="pg0")
                for i in range(14):
                    nc.tensor.matmul(wut[:, 0:512], wsrc0[:, 0:128],
                                     wsrc0[:], start=True, stop=True)
                if not ln_trivial:
                    gq_s = wpool.tile([128, 4, D], f32, name="gq_s")
                    nc.sync.dma_start(gq_s[:], gqb[:])
                    bq_s = wpool.tile([128, 4, D], f32, name="bq_s")
                    nc.sync.dma_start(bq_s[:], bqb[:])
                    gk_s = wpool.tile([128, 4, D], f32, name="gk_s")
                    nc.sync.dma_start(gk_s[:], gkb[:])
                    bk_s = wpool.tile([128, 4, D], f32, name="bk_s")
                    nc.sync.dma_start(bk_s[:], bkb[:])

                # ---- phases 1-3: qkv matmuls kc-outer over t-groups of 3
                # (PSUM: 3 groups x [128,768] = 6 banks + 2 transpose banks)
                q_nat = wpool.tile([128, NT, 4, D], bf, name="q_nat")
                k_nat = wpool.tile([128, NT, 4, D], bf, name="k_nat")

                s1q = wpool.tile([128, NT, 4], bf, name="s1q")
                s2q = wpool.tile([128, NT, 4], bf, name="s2q")
                s1k = wpool.tile([128, NT, 4], bf, name="s1k")
                s2k = wpool.tile([128, NT, 4], bf, name="s2k")
                mu_q = bpool.tile([128, NT, 4], f32, name="mu_q")
                rs_q = bpool.tile([128, NT, 4], f32, name="rs_q")
                mu_k = bpool.tile([128, NT, 4], f32, name="mu_k")
                rs_k = bpool.tile([128, NT, 4], f32, name="rs_k")
                # rs_k pre-scaled for the DVE Schraudolph exp path
                rs_sch = bpool.tile([128, NT, 4], f32, name="rs_sch")

                groups = [list(range(g, min(g + 3, NT))) for g in range(0, NT, 3)]
                applied = 0
                tbatch = 0

                def emit_transpose_batch(b):
                    # transposes for t = 4b .. 4b+3 (k first: no apply dep)
                    cols = slice(b * 512, (b + 1) * 512)
                    for s in range(2):
                        ptp = tps.tile([128, 4, 128], bf, tag="ptp")
                        for i in range(4):
                            t = 4 * b + i
                            nc.tensor.transpose(
                                ptp[:, i], k_nat[:, t, 2 * s:2 * s + 2, :],
                                ident[:])
                        # split into the two zero-padded per-head tiles
                        dst0 = kTz[2 * s][0:64, cols].rearrange(
                            "p (a b) -> p a b", a=4)
                        dst1 = kTz[2 * s + 1][64:128, cols].rearrange(
                            "p (a b) -> p a b", a=4)
                        if b % 2 == 0:
                            nc.vector.tensor_copy(dst0, ptp[0:64])
                            nc.scalar.copy(dst1, ptp[64:128])
                        else:
                            nc.scalar.copy(dst0, ptp[0:64])
                            nc.vector.tensor_copy(dst1, ptp[64:128])
                    for s in range(2):
                        ptp = tps.tile([128, 4, 128], bf, tag="ptp")
                        for i in range(4):
                            t = 4 * b + i
                            nc.tensor.transpose(
                                ptp[:, i], q_nat[:, t, 2 * s:2 * s + 2, :],
                                ident[:])
                        if b % 2 == 0:
                            nc.vector.tensor_copy(qT[s][:, cols], ptp[:])
                        else:
                            nc.scalar.copy(qT[s][:, cols], ptp[:])

                for gi, grp in enumerate(groups):
                    pq_t = {}
                    for t in grp:
                        pq_t[t] = qps.tile([128, 3 * DPC], f32,
                                           tag=f"pg{t % 3}", name=f"pqkv{t}")
                    for kc in range(KT):
                        for t in grp:
                            ts_ = slice(t * 128, (t + 1) * 128)
                            lhs = xt_s[:, kc, ts_]
                            st = kc == 0
                            sp = kc == KT - 1
                            nc.tensor.matmul(pq_t[t][:, 0:512], lhs,
                                             wqkv_s[:, kc, 0:512],
                                             start=st, stop=sp)
                            nc.tensor.matmul(pq_t[t][:, 512:768], lhs,
                                             wqkv_s[:, kc, 512:768],
                                             start=st, stop=sp)
                    # evacuate PSUM on ACT, then all stats on DVE from the
                    # bf16 SBUF copies (2x/4x DVE modes, lighter PSUM load)
                    for t in grp:
                        pall = pq_t[t]
                        for (lo, natd) in ((0, k_nat), (256, q_nat)):
                            pg = pall[:, lo:lo + 256].rearrange(
                                "p (g d) -> p g d", g=4)
                            nc.scalar.copy(natd[:, t], pg)
                        nc.scalar.copy(v5[:, t * HPC:(t + 1) * HPC, 0:64],
                                       pall[:, 512:768].rearrange(
                                           "p (g d) -> p g d", g=4))
                    gs = slice(grp[0], grp[-1] + 1)
                    ng = len(grp)
                    with nc.allow_low_precision("bf16 LN stats (D=64 sums)"):
                        for (s1, s2, natd) in ((s1k, s2k, k_nat),
                                               (s1q, s2q, q_nat)):
                            nc.vector.tensor_reduce(s1[:, gs], natd[:, gs],
                                                    AX.X, OP.add)
                            sq = sqpool.tile([128, ng, 4, D], bf,
                                             tag=f"sq{ng}")
                            nc.vector.tensor_mul(sq[:], natd[:, gs],
                                                 natd[:, gs])
                            nc.vector.tensor_reduce(s2[:, gs], sq[:],
                                                    AX.X, OP.add)

                    # stats finalize for this group
                    # mu = s1/64 ; var = s2/64 - mu^2 ; rstd = 1/sqrt(var+eps)
                    # Trivial-LN k path: rs_k holds SCALE*rstd (the LN mean
                    # subtraction cancels against zero-mean q-hat in the
                    # scores, and rstd*SCALE is applied as the per-partition
                    # exp scale), so k needs no apply at all.
                    for (s1, s2, mu, rs, kfold) in (
                            (s1k, s2k, mu_k, rs_k, ln_trivial),
                            (s1q, s2q, mu_q, rs_q, False)):
                        nc.vector.tensor_scalar(mu[:, gs], s1[:, gs], 1.0 / D,
                                                None, OP.mult)
                        u = stp.tile([128, ng, 4], f32, tag=f"u{ng}")
                        nc.vector.scalar_tensor_tensor(u[:], s1[:, gs], 1.0 / D,
                                                       s1[:, gs], OP.mult, OP.mult)
                        u2 = stp.tile([128, ng, 4], f32, tag=f"u2{ng}")
                        nc.vector.scalar_tensor_tensor(u2[:], u[:], -1.0,
                                                       s2[:, gs], OP.mult, OP.add)
                        if kfold:
                            # rs = 1/sqrt(64*var + 64*eps) = SCALE/sqrt(var+eps)
                            nc.scalar.activation(u[:], u2[:], ACTF.Sqrt,
                                                 bias=eps64_t[:], scale=1.0)
                        else:
                            nc.scalar.activation(u[:], u2[:], ACTF.Sqrt,
                                                 bias=eps_t[:], scale=1.0 / D)
                        nc.vector.reciprocal(rs[:, gs], u[:])
                    if ln_trivial:
                        nc.vector.tensor_scalar(rs_sch[:, gs], rs_k[:, gs],
                                                SCH_A, None, OP.mult)

                    # apply for this group (overlaps next group's qkv on PE)
                    # broadcast mu/rs along the 64 head-dims via stride-0 APs
                    apply_list = ([(q_nat, mu_q, rs_q)] if ln_trivial else
                                  [(k_nat, mu_k, rs_k), (q_nat, mu_q, rs_q)])

                    def bcast_d(ap):
                        return bass.AP(tensor=ap.tensor, offset=ap.offset,
                                       ap=list(ap.ap) + [[0, D]])

                    for (nat, mu, rs) in apply_list:
                        nc.vector.tensor_sub(nat[:, gs], nat[:, gs],
                                             bcast_d(mu[:, gs]))
                        nc.vector.tensor_mul(nat[:, gs], nat[:, gs],
                                             bcast_d(rs[:, gs]))
                        if not ln_trivial:
                            gb = gq_s if nat is q_nat else gk_s
                            bb = bq_s if nat is q_nat else bk_s
                            for t in grp:
                                nc.vector.tensor_mul(nat[:, t], nat[:, t], gb[:])
                                nc.vector.tensor_add(nat[:, t], nat[:, t], bb[:])
                    while (tbatch + 1) * 4 <= applied:
                        emit_transpose_batch(tbatch)
                        tbatch += 1
                    applied = grp[-1] + 1
                while tbatch < 4:
                    emit_transpose_batch(tbatch)
                    tbatch += 1

                if not ln_trivial:
                    # center k over sequence (softmax-invariant, kept only
                    # for the general gamma/beta path)
                    with tc.tile_pool(name="ctr", bufs=1) as ctr:
                        for h in range(4):
                            off = 64 * (h % 2)
                            rows = slice(off, off + 64)
                            rsum = ctr.tile([128, 1], f32, tag="rsum")
                            nc.vector.tensor_reduce(rsum[0:64], kTz[h][rows],
                                                    AX.X, OP.add)
                            mean = ctr.tile([128, 1], f32, tag="mean")
                            nc.vector.tensor_scalar(mean[0:64], rsum[0:64],
                                                    1.0 / N, None, OP.mult)
                            nc.vector.tensor_scalar(kTz[h][rows], kTz[h][rows],
                                                    mean[0:64], None,
                                                    OP.subtract)

            # ---- attention + deferred normalize + interleaved proj ----
            # Normalization is deferred: during the head loop only raw U
            # and the denominator row are evacuated, keeping PE dense.
            # Chunks run nh-MAJOR so the nh=0 half of normalize+proj+DMA
            # overlaps the nh=1 attention chunks.
            with tc.tile_pool(name="exp_pool", bufs=4) as epool, \
                 tc.tile_pool(name="nrm_pool", bufs=1) as npool, \
                 tc.tile_pool(name="fin", bufs=4) as fpool, \
                 tc.tile_pool(name="att_ps", bufs=1, space="PSUM") as aps:
                den_all = npool.tile([1, 2, HPC, 1024], f32, name="den_all")
                denr = npool.tile([1, 2, HPC, 1024], fr, name="denr")
                # HAM warm-up: half-array matmuls (K=64 scores / M=65 attnv)
                # never un-throttle the PE clock from cold; a short burst of
                # full-array matmuls brings it to 2.4 GHz before the head loop.
                wps = aps.tile([128, 2, 512], f32, tag="ps", bufs=2)
                for i in range(10):
                    nc.tensor.matmul(wps[:, i % 2], qT[0][:, 0:128],
                                     qT[0][:, 0:512], start=True, stop=True)
                # nh-major chunk order; flat sequence with a lag-2 pipeline
                # ACROSS chunk boundaries so ACT/DVE never drain at
                # transitions. U stays single-buffered.
                chunks = [(h, nh) for nh in range(2) for h in range(HPC)]
                seq = [(ci, mt) for ci in range(len(chunks)) for mt in range(NT)]
                Us = {}
                exs = {}

                def emit_attnv(ci, mt):
                    h, nh = chunks[ci]
                    exv = exs.pop((ci, mt))
                    for j in range(2):
                        nc.tensor.matmul(Us[ci][:, j * 512:(j + 1) * 512],
                                         v5[:, mt * HPC + h, :],
                                         exv[:, j * 512:(j + 1) * 512],
                                         start=(mt == 0), stop=(mt == NT - 1))
                    if mt == NT - 1:
                        p = h // 2
                        off = 64 * (h % 2)
                        nc.vector.tensor_copy(
                            outT[p][off:off + 64, nh * 1024:(nh + 1) * 1024],
                            Us[ci][0:64, :])
                        nc.vector.tensor_copy(den_all[:, nh, h], Us[ci][64:65, :])
                        del Us[ci]

                def emit_recip(nh, hs=slice(0, HPC)):
                    # reciprocal of denominators: DMA-reshape to 128
                    # partitions so the iterative divide runs on few
                    # elems/lane instead of thousands on one lane.
                    nhp = (hs.stop - hs.start) * 8
                    den128 = npool.tile([128, 32], f32, tag="den128", bufs=3)
                    nc.sync.dma_start(
                        den128[:, 0:nhp],
                        den_all[:, nh, hs].rearrange("o h f -> o (h f)"))
                    der128 = npool.tile([128, 32], fr, tag="der128", bufs=3)
                    with nc.allow_low_precision("softmax recip"):
                        nc.vector.reciprocal(der128[:, 0:nhp], den128[:, 0:nhp])
                    nc.gpsimd.dma_start(
                        denr[:, nh, hs].rearrange("o h f -> o (h f)"),
                        der128[:, 0:nhp])

                def emit_recip_act(nh, h):
                    # 1/den = exp(-ln(den)) on ACT: skips the DMA reshape
                    # round-trip; used for the last head where the DMA
                    # latency would stall the projection tail
                    lnb = npool.tile([1, 1024], f32, tag="lnb", bufs=2)
                    nc.scalar.activation(lnb[:], den_all[:, nh, h], ACTF.Ln)
                    nc.scalar.activation(denr[:, nh, h], lnb[:], ACTF.Exp,
                                         scale=-1.0)

                def emit_norm(nh, h):
                    # broadcast 1/den across the 64 head-dim rows via a
                    # K=1 ones matmul, then normalize outT on DVE
                    p = h // 2
                    off = 64 * (h % 2)
                    rt = aps.tile([128, 2, 512], f32, tag="ps", bufs=2,
                                  name=f"rt{nh}{h}")
                    rbp = rt[0:64].rearrange("p a b -> p (a b)")
                    for j in range(2):
                        nc.tensor.matmul(rbp[:, j * 512:(j + 1) * 512],
                                         ones_t[:],
                                         denr[:, nh, h, j * 512:(j + 1) * 512],
                                         start=True, stop=True)
                    sl = outT[p][off:off + 64, nh * 1024:(nh + 1) * 1024]
                    nc.vector.tensor_mul(sl, sl, rbp[:])

                def emit_proj(trange):
                    for t in trange:
                        ts_ = slice(t * 128, (t + 1) * 128)
                        po = aps.tile([128, 2, 512], f32, tag="ps", bufs=2)
                        for p in range(2):
                            for j in range(2):
                                nc.tensor.matmul(po[:, j],
                                                 outT[p][:, ts_],
                                                 wp_s[:, p, j * 512:(j + 1) * 512],
                                                 start=(p == 0), stop=(p == 1))
                        fin = fpool.tile([128, 1024], f32, tag="fin")
                        if t % 2 == 0:
                            nc.vector.tensor_copy(
                                fin[:], po[:].rearrange("p a b -> p (a b)"))
                        else:
                            nc.scalar.copy(
                                fin[:], po[:].rearrange("p a b -> p (a b)"))
                        eng = (nc.sync, nc.gpsimd, nc.scalar)[t % 3]
                        eng.dma_start(out[ts_, :], fin[:])

                inserts = {
                    (4, 3): lambda: emit_recip(0),
                    (5, 1): lambda: emit_norm(0, 0),
                    (5, 6): lambda: emit_norm(0, 1),
                    (5, 11): lambda: emit_norm(0, 2),
                    (6, 0): lambda: emit_norm(0, 3),
                    (6, 5): lambda: emit_proj(range(0, 2)),
                    (6, 10): lambda: emit_proj(range(2, 4)),
                    (6, 15): lambda: emit_recip(1, slice(0, 2)),
                    (7, 2): lambda: emit_recip(1, slice(2, 3)),
                    (7, 5): lambda: emit_norm(1, 0),
                    (7, 8): lambda: emit_norm(1, 1),
                    (7, 11): lambda: emit_norm(1, 2),
                    (7, 14): lambda: emit_proj(range(4, 8)),
                }

                for i, (ci, mt) in enumerate(seq):
                    h, nh = chunks[ci]
                    if mt == 0:
                        Us[ci] = aps.tile([128, 1024], f32, tag="U", bufs=2,
                                          name=f"U{ci}")
                    p = h // 2
                    off = 64 * (h % 2)
                    ms = slice(mt * 128, (mt + 1) * 128)
                    ps = aps.tile([128, 2, 512], f32, tag="ps", bufs=2)
                    for j in range(2):
                        ns = slice(nh * 1024 + j * 512,
                                   nh * 1024 + (j + 1) * 512)
                        nc.tensor.matmul(ps[:, j], kTz[h][:, ms],
                                         qT[p][:, ns],
                                         start=True, stop=True)
                    ex = epool.tile([128, 1024], bf, tag="ex", bufs=4)
                    psv = ps[:].rearrange("p a b -> p (a b)")
                    if ln_trivial:
                        if mt in DVE_MTS:
                            nc.vector.tensor_scalar(
                                ex[:].bitcast(i16), psv,
                                rs_sch[:, mt, h:h + 1], SCH_B,
                                OP.mult, OP.add)
                        else:
                            nc.scalar.activation(ex[:], psv, ACTF.Exp,
                                                 scale=rs_k[:, mt, h:h + 1])
                    else:
                        if mt in DVE_MTS:
                            nc.vector.tensor_scalar(
                                ex[:].bitcast(i16), psv,
                                SCALE * SCH_A, SCH_B, OP.mult, OP.add)
                        else:
                            nc.scalar.activation(ex[:], psv, ACTF.Exp,
                                                 scale=SCALE)
                    exs[(ci, mt)] = ex
                    if i >= 2:
                        emit_attnv(*seq[i - 2])
                    cb = inserts.pop((ci, mt), None)
                    if cb is not None:
                        cb()
                for i in (len(seq) - 2, len(seq) - 1):
                    emit_attnv(*seq[i])

                # ---- nh=1 tail: recip/normalize/proj + output DMA -----
                emit_recip_act(1, 3)
                emit_norm(1, 3)
                emit_proj(range(8, 16))

    return nc


def _get_program(ln_trivial: bool):
    key = ln_trivial
    if key not in _nc_cache:
        _install_patch()
        _nc_cache[key] = _build_program(ln_trivial)
    return _nc_cache[key]


def _bf16():
    import ml_dtypes
    return ml_dtypes.bfloat16


def _prep_core_inputs(c, x, qkv_w, q_norm_w, q_norm_b, k_norm_w, k_norm_b,
                      proj_w, ln_trivial):
    b = c // 4
    g = c % 4
    rows = slice(g * DPC, (g + 1) * DPC)
    b16 = _bf16()
    xt = np.ascontiguousarray(x[b].T).astype(b16)           # [C, N]
    wk = qkv_w[C:2 * C, :][rows, :].T                        # [C, DPC]
    wq = qkv_w[rows, :].T
    wv = qkv_w[2 * C:3 * C, :][rows, :].T
    wqkv = np.ascontiguousarray(
        np.concatenate([wk, wq, wv], axis=1)).astype(b16)    # [C, 3*DPC]
    wp = np.ascontiguousarray(proj_w[:, rows].T).astype(b16)  # [DPC, C]
    m = {"xt": xt, "wqkv": wqkv, "wp": wp,
         "ident": np.eye(128, dtype=_bf16()),
         "ones64": np.ones((1, D), dtype=np.float32)}
    if not ln_trivial:
        for nm, arr in (("gqb", q_norm_w), ("bqb", q_norm_b),
                        ("gkb", k_norm_w), ("bkb", k_norm_b)):
            t = np.broadcast_to(arr.astype(np.float32), (128, 4, D))
            m[nm] = np.ascontiguousarray(t)
    return m


def kernel(x, qkv_w, q_norm_w, q_norm_b, k_norm_w, k_norm_b, proj_w, proj_b,
           _trace=False):
    from concourse.bass_utils import run_bass_kernel_spmd

    x = np.asarray(x, dtype=np.float32)
    qkv_w = np.asarray(qkv_w, dtype=np.float32)
    q_norm_w = np.asarray(q_norm_w, dtype=np.float32)
    q_norm_b = np.asarray(q_norm_b, dtype=np.float32)
    k_norm_w = np.asarray(k_norm_w, dtype=np.float32)
    k_norm_b = np.asarray(k_norm_b, dtype=np.float32)
    proj_w = np.asarray(proj_w, dtype=np.float32)
    proj_b = np.asarray(proj_b, dtype=np.float32)

    ln_trivial = (np.all(q_norm_w == 1.0) and np.all(q_norm_b == 0.0)
                  and np.all(k_norm_w == 1.0) and np.all(k_norm_b == 0.0))

    nc = _get_program(ln_trivial)
    in_maps = [
        _prep_core_inputs(c, x, qkv_w, q_norm_w, q_norm_b, k_norm_w,
                          k_norm_b, proj_w, ln_trivial)
        for c in range(NCORES)
    ]
    res = run_bass_kernel_spmd(nc, in_maps, list(range(NCORES)),
                               trace=_trace)
    outs = [res.results[c]["out"] for c in range(NCORES)]
    full = np.empty((B, N, C), dtype=np.float32)
    for b in range(B):
        acc = outs[4 * b].astype(np.float32)
        for g in range(1, 4):
            acc = acc + outs[4 * b + g]
        full[b] = acc + proj_b[None, :]
    if _trace:
        return full, res
    return full


# revision 20
# speedup vs baseline: 1.0521x; 1.0521x over previous
"""Trainium2 Bass kernel for nn_Attention_32091995636193.

Dense transformer attention block:
  qkv = x @ qkv_w.T ; per-head LN(q), LN(k) over head_dim ; k centered over
  seq ; softmax(q*scale @ k^T) @ v ; out @ proj_w.T + proj_b.

Sharding over 8 NeuronCores: data parallel on batch (B=2) x tensor parallel
on heads (16 heads -> 4 per core). Core c handles batch c//4, heads
4*(c%4) .. 4*(c%4)+3. Each core computes its partial projection output
[N, C]; the host sums the 4 partials per batch and adds proj_b.

Per-core device program:
  1. qkv matmuls kc-OUTER over t-groups of 3 so PE starts as soon as the
     first xt k-tile DMA lands (combined [wk|wq|wv] weight, 2 matmuls/t/kc)
  2. LN stats batched per group on bf16 SBUF copies (DVE 2x modes);
     broadcast-AP apply; PE-transpose in t-batches of 4
  3. attention chunks in nh-MAJOR order. ALL attention matmuls run
     full-array (scores use zero-padded per-head kTz so K=128; v5 is
     padded to M=128): half-array matmuls keep the PE HAM clock-gate
     throttled at ~1.8GHz, full-array runs at 2.4GHz. exp mostly on ACT
     with a 4/16 Schraudolph (int16-bitcast bf16) split on DVE;
     denominator via ones-row in the attnv matmul
  4. deferred normalization: reciprocal via 128-partition DMA reshape
     (ACT ln/exp chain for the last head to skip the DMA latency),
     K=1 ones-matmul broadcast, DVE multiply - interleaved into the
     attention chunk stream per head as denominators become ready
  5. projection + output DMA interleaved into chunks 6-7; only the
     nh=1 second half of proj remains as tail
"""

import os
import sys

for _p in ("/opt/trn_rl_repo",):
    if _p not in sys.path and os.path.isdir(_p):
        sys.path.append(_p)

import numpy as np

# ---------------------------------------------------------------------------
# BIR legalizer: the pinned walrus build supports at most ONE sync wait per
# instruction, but Tile's scheduler attaches several. Split extra waits onto
# NoOp instructions inserted immediately before (same engine => same NX
# order => identical semantics).
# ---------------------------------------------------------------------------
import orjson


def _legalize_bir_json_bytes(raw: bytes) -> bytes:
    j = orjson.loads(raw)
    counter = 0
    for f in j.get("functions", []):
        for blk in f.get("blocks", []):
            insts = blk.get("instructions")
            if not insts:
                continue
            out = []
            for ins in insts:
                si = ins.get("sync_info")
                waits = si.get("on_wait") if si else None
                if waits and len(waits) > 1:
                    engine = ins.get("engine")
                    for w in waits[:-1]:
                        counter += 1
                        nop = {
                            "name": f"WSPLIT-{counter}",
                            "opcode": "NoOp",
                            "engine": engine,
                            "ins": [],
                            "outs": [],
                            "sync_info": {"on_wait": [w], "on_update": []},
                        }
                        if "debug" in ins:
                            nop["debug"] = ins["debug"]
                        out.append(nop)
                    si["on_wait"] = [waits[-1]]
                out.append(ins)
            blk["instructions"] = out
    return orjson.dumps(j)


_PATCHED = False


def _install_patch():
    global _PATCHED
    if _PATCHED:
        return
    import concourse.bass as bass

    orig = bass.Bass.to_json_bytes

    def patched(self):
        return _legalize_bir_json_bytes(orig(self))

    bass.Bass.to_json_bytes = patched
    _PATCHED = True


# ---------------------------------------------------------------------------
# Problem constants (hardcoded per the harness contract)
# ---------------------------------------------------------------------------
B = 2
N = 2048
C = 1024
H = 16
D = 64
SCALE = D ** -0.5
EPS = 1e-5
NCORES = 8
HPC = H // 4          # heads per core = 4
DPC = HPC * D         # channels per core = 256
NT = N // 128         # 16 n-tiles
KT = C // 128         # 8 contraction tiles

# Schraudolph exp approximation in bf16 (int16 bitcast):
#   exp(x) ~ bitcast_bf16(int16(round(x * 128/ln2 + (127 - c)*128)))
SCH_A = 128.0 / np.log(2.0)          # 184.6650
SCH_B = (127.0 - 0.04367) * 128.0    # 16250.41
# mt indices whose exp runs on DVE instead of ACT (per chunk)
DVE_MTS = frozenset((1, 4, 7, 10, 13))

_nc_cache = {}


def _build_program(ln_trivial: bool):
    import concourse.bass as bass
    import concourse.mybir as mybir
    import concourse.tile as tile

    fr = mybir.dt.float32r
    f32 = mybir.dt.float32
    bf = mybir.dt.bfloat16
    i16 = mybir.dt.int16
    AX = mybir.AxisListType
    OP = mybir.AluOpType
    ACTF = mybir.ActivationFunctionType

    nc = bass.Bass()
    xt = nc.declare_dram_parameter("xt", [C, N], bf, isOutput=False)
    # combined qkv weight slice, column order [wk | wq | wv] (DPC each)
    wqkv = nc.declare_dram_parameter("wqkv", [C, 3 * DPC], bf, isOutput=False)
    wp = nc.declare_dram_parameter("wp", [DPC, C], bf, isOutput=False)
    ones64 = nc.declare_dram_parameter("ones64", [1, D], fr, isOutput=False)
    ident_in = nc.declare_dram_parameter("ident", [128, 128], bf, isOutput=False)
    if not ln_trivial:
        gqb = nc.declare_dram_parameter("gqb", [128, 4, D], f32, isOutput=False)
        bqb = nc.declare_dram_parameter("bqb", [128, 4, D], f32, isOutput=False)
        gkb = nc.declare_dram_parameter("gkb", [128, 4, D], f32, isOutput=False)
        bkb = nc.declare_dram_parameter("bkb", [128, 4, D], f32, isOutput=False)
    out = nc.declare_dram_parameter("out", [N, C], f32, isOutput=True)

    with tile.TileContext(nc) as tc:
        with tc.tile_pool(name="const", bufs=1) as cpool, \
             tc.tile_pool(name="persist", bufs=1) as bpool:

            ident = cpool.tile([128, 128], bf)
            ones_t = cpool.tile([1, D], fr)
            wsrc0 = cpool.tile([128, 512], bf)
            nc.vector.memset(wsrc0[:], 0.01)
            eps_t = cpool.tile([128, 1], f32)
            nc.vector.memset(eps_t[:], EPS)
            eps64_t = cpool.tile([128, 1], f32)
            nc.vector.memset(eps64_t[:], D * EPS)

            # ---- persistent tensors (live into attention/proj) --------
            wp_s = bpool.tile([128, 2, C], bf, name="wp_s")
            v5 = bpool.tile([128, NT * HPC, 128], bf, name="v5")
            nc.gpsimd.memset(v5[:, :, 64:65], 1.0)
            nc.gpsimd.memset(v5[:, :, 65:128], 0.0)
            qT = [bpool.tile([128, N], bf, name=f"qT{p}") for p in range(2)]
            kTz = [bpool.tile([128, N], bf, name=f"kTz{h}") for h in range(4)]
            for h in range(4):
                off = 64 * (h % 2)
                nc.vector.memset(kTz[h][64 - off:128 - off, :], 0.0)
            outT = [bpool.tile([128, N], bf, name=f"outT{p}") for p in range(2)]

            with tc.tile_pool(name="ph13", bufs=1) as wpool, \
                 tc.tile_pool(name="qkv_ps", bufs=1, space="PSUM") as qps, \
                 tc.tile_pool(name="sq_pool", bufs=2) as sqpool, \
                 tc.tile_pool(name="stat_tmp", bufs=2) as stp, \
                 tc.tile_pool(name="tp_ps", bufs=2, space="PSUM") as tps:
                # ---- load inputs/weights for phases 1-3 ---------------
                # per-k-tile DMAs, interleaved weight/xt so the kc-outer
                # accumulation can start as soon as k-tile 0 lands
                wqkv_s = wpool.tile([128, KT, 3 * DPC], bf, name="wqkv_s")
                wqkv_r = wqkv.rearrange("(k p) d -> p k d", p=128)
                xt_s = wpool.tile([128, KT, N], bf, name="xt_s")
                xt_r = xt.rearrange("(k p) n -> p k n", p=128)
                for kc in range(KT):
                    eng = (nc.sync, nc.scalar, nc.gpsimd)[kc % 3]
                    eng.dma_start(xt_s[:, kc:kc + 1], xt_r[:, kc:kc + 1])
                    eng.dma_start(wqkv_s[:, kc:kc + 1], wqkv_r[:, kc:kc + 1])
                # late-needed loads at the tail of the queues
                nc.sync.dma_start(ident[:], ident_in[:])
                nc.scalar.dma_start(ones_t[:], ones64[:])
                nc.gpsimd.dma_start(wp_s[:], wp.rearrange("(k p) n -> p k n", p=128))
                # warm the PE clock from a memset scratch: starts right at
                # program begin, no DMA dependency
                # (borrows the pg0 psum slot; W-after-W serializes safely)
                wut = qps.tile([128, 3 * DPC], f32, tag="pg0")
                for i in range(14):
                    nc.tensor.matmul(wut[:, 0:512], wsrc0[:, 0:128],
                                     wsrc0[:], start=True, stop=True)
                if not ln_trivial:
                    gq_s = wpool.tile([128, 4, D], f32, name="gq_s")
                    nc.sync.dma_start(gq_s[:], gqb[:])
                    bq_s = wpool.tile([128, 4, D], f32, name="bq_s")
                    nc.sync.dma_start(bq_s[:], bqb[:])
                    gk_s = wpool.tile([128, 4, D], f32, name="gk_s")
                    nc.sync.dma_start(gk_s[:], gkb[:])
                    bk_s = wpool.tile([128, 4, D], f32, name="bk_s")
                    nc.sync.dma_start(bk_s[:], bkb[:])

                # ---- phases 1-3: qkv matmuls kc-outer over t-groups of 3
                # (PSUM: 3 groups x [128,768] = 6 banks + 2 transpose banks)
                q_nat = wpool.tile([128, NT, 4, D], bf, name="q_nat")
                k_nat = wpool.tile([128, NT, 4, D], bf, name="k_nat")

                s1q = wpool.tile([128, NT, 4], bf, name="s1q")
                s2q = wpool.tile([128, NT, 4], bf, name="s2q")
                s1k = wpool.tile([128, NT, 4], bf, name="s1k")
                s2k = wpool.tile([128, NT, 4], bf, name="s2k")
                mu_q = bpool.tile([128, NT, 4], f32, name="mu_q")
                rs_q = bpool.tile([128, NT, 4], f32, name="rs_q")
                mu_k = bpool.tile([128, NT, 4], f32, name="mu_k")
                rs_k = bpool.tile([128, NT, 4], f32, name="rs_k")
                # rs_k pre-scaled for the DVE Schraudolph exp path
                rs_sch = bpool.tile([128, NT, 4], f32, name="rs_sch")

                groups = [list(range(g, min(g + 3, NT))) for g in range(0, NT, 3)]
                applied = 0
                tbatch = 0

                def emit_transpose_batch(b):
                    # transposes for t = 4b .. 4b+3 (k first: no apply dep)
                    cols = slice(b * 512, (b + 1) * 512)
                    for s in range(2):
                        ptp = tps.tile([128, 4, 128], bf, tag="ptp")
                        for i in range(4):
                            t = 4 * b + i
                            nc.tensor.transpose(
                                ptp[:, i], k_nat[:, t, 2 * s:2 * s + 2, :],
                                ident[:])
                        # split into the two zero-padded per-head tiles
                        dst0 = kTz[2 * s][0:64, cols].rearrange(
                            "p (a b) -> p a b", a=4)
                        dst1 = kTz[2 * s + 1][64:128, cols].rearrange(
                            "p (a b) -> p a b", a=4)
                        if b % 2 == 0:
                            nc.vector.tensor_copy(dst0, ptp[0:64])
                            nc.scalar.copy(dst1, ptp[64:128])
                        else:
                            nc.scalar.copy(dst0, ptp[0:64])
                            nc.vector.tensor_copy(dst1, ptp[64:128])
                    for s in range(2):
                        ptp = tps.tile([128, 4, 128], bf, tag="ptp")
                        for i in range(4):
                            t = 4 * b + i
                            nc.tensor.transpose(
                                ptp[:, i], q_nat[:, t, 2 * s:2 * s + 2, :],
                                ident[:])
                        if b % 2 == 0:
                            nc.vector.tensor_copy(qT[s][:, cols], ptp[:])
                        else:
                            nc.scalar.copy(qT[s][:, cols], ptp[:])

                for gi, grp in enumerate(groups):
                    pq_t = {}
                    for t in grp:
                        pq_t[t] = qps.tile([128, 3 * DPC], f32,
                                           tag=f"pg{t % 3}", name=f"pqkv{t}")
                    for kc in range(KT):
                        for t in grp:
                            ts_ = slice(t * 128, (t + 1) * 128)
                            lhs = xt_s[:, kc, ts_]
                            st = kc == 0
                            sp = kc == KT - 1
                            nc.tensor.matmul(pq_t[t][:, 0:512], lhs,
                                             wqkv_s[:, kc, 0:512],
                                             start=st, stop=sp)
                            nc.tensor.matmul(pq_t[t][:, 512:768], lhs,
                                             wqkv_s[:, kc, 512:768],
                                             start=st, stop=sp)
                    # evacuate PSUM on ACT, then all stats on DVE from the
                    # bf16 SBUF copies (2x/4x DVE modes, lighter PSUM load)
                    for t in grp:
                        pall = pq_t[t]
                        for (lo, natd) in ((0, k_nat), (256, q_nat)):
                            pg = pall[:, lo:lo + 256].rearrange(
                                "p (g d) -> p g d", g=4)
                            nc.scalar.copy(natd[:, t], pg)
                        nc.scalar.copy(v5[:, t * HPC:(t + 1) * HPC, 0:64],
                                       pall[:, 512:768].rearrange(
                                           "p (g d) -> p g d", g=4))
                    gs = slice(grp[0], grp[-1] + 1)
                    ng = len(grp)
                    with nc.allow_low_precision("bf16 LN stats (D=64 sums)"):
                        for (s1, s2, natd) in ((s1k, s2k, k_nat),
                                               (s1q, s2q, q_nat)):
                            nc.vector.tensor_reduce(s1[:, gs], natd[:, gs],
                                                    AX.X, OP.add)
                            sq = sqpool.tile([128, ng, 4, D], bf,
                                             tag=f"sq{ng}")
                            nc.vector.tensor_mul(sq[:], natd[:, gs],
                                                 natd[:, gs])
                            nc.vector.tensor_reduce(s2[:, gs], sq[:],
                                                    AX.X, OP.add)

                    # stats finalize for this group
                    # mu = s1/64 ; var = s2/64 - mu^2 ; rstd = 1/sqrt(var+eps)
                    # Trivial-LN k path: rs_k holds SCALE*rstd (the LN mean
                    # subtraction cancels against zero-mean q-hat in the
                    # scores, and rstd*SCALE is applied as the per-partition
                    # exp scale), so k needs no apply at all.
                    for (s1, s2, mu, rs, kfold) in (
                            (s1k, s2k, mu_k, rs_k, ln_trivial),
                            (s1q, s2q, mu_q, rs_q, False)):
                        nc.vector.tensor_scalar(mu[:, gs], s1[:, gs], 1.0 / D,
                                                None, OP.mult)
                        u = stp.tile([128, ng, 4], f32, tag=f"u{ng}")
                        nc.vector.scalar_tensor_tensor(u[:], s1[:, gs], 1.0 / D,
                                                       s1[:, gs], OP.mult, OP.mult)
                        u2 = stp.tile([128, ng, 4], f32, tag=f"u2{ng}")
                        nc.vector.scalar_tensor_tensor(u2[:], u[:], -1.0,
                                                       s2[:, gs], OP.mult, OP.add)
                        if kfold:
                            # rs = 1/sqrt(64*var + 64*eps) = SCALE/sqrt(var+eps)
                            nc.scalar.activation(u[:], u2[:], ACTF.Sqrt,
                                                 bias=eps64_t[:], scale=1.0)
                        else:
                            nc.scalar.activation(u[:], u2[:], ACTF.Sqrt,
                                                 bias=eps_t[:], scale=1.0 / D)
                        nc.vector.reciprocal(rs[:, gs], u[:])
                    if ln_trivial:
                        nc.vector.tensor_scalar(rs_sch[:, gs], rs_k[:, gs],
                                                SCH_A, None, OP.mult)

                    # apply for this group (overlaps next group's qkv on PE)
                    # broadcast mu/rs along the 64 head-dims via stride-0 APs
                    apply_list = ([(q_nat, mu_q, rs_q)] if ln_trivial else
                                  [(k_nat, mu_k, rs_k), (q_nat, mu_q, rs_q)])

                    def bcast_d(ap):
                        return bass.AP(tensor=ap.tensor, offset=ap.offset,
                                       ap=list(ap.ap) + [[0, D]])

                    for (nat, mu, rs) in apply_list:
                        nc.vector.tensor_sub(nat[:, gs], nat[:, gs],
                                             bcast_d(mu[:, gs]))
                        nc.vector.tensor_mul(nat[:, gs], nat[:, gs],
                                             bcast_d(rs[:, gs]))
                        if not ln_trivial:
                            gb = gq_s if nat is q_nat else gk_s
                            bb = bq_s if nat is q_nat else bk_s
                            for t in grp:
                                nc.vector.tensor_mul(nat[:, t], nat[:, t], gb[:])
                                nc.vector.tensor_add(nat[:, t], nat[:, t], bb[:])
                    while (tbatch + 1) * 4 <= applied:
                        emit_transpose_batch(tbatch)
                        tbatch += 1
                    applied = grp[-1] + 1
                while tbatch < 4:
                    emit_transpose_batch(tbatch)
                    tbatch += 1

                if not ln_trivial:
                    # center k over sequence (softmax-invariant, kept only
                    # for the general gamma/beta path)
                    with tc.tile_pool(name="ctr", bufs=1) as ctr:
                        for h in range(4):
                            off = 64 * (h % 2)
                            rows = slice(off, off + 64)
                            rsum = ctr.tile([128, 1], f32, tag="rsum")
                            nc.vector.tensor_reduce(rsum[0:64], kTz[h][rows],
                                                    AX.X, OP.add)
                            mean = ctr.tile([128, 1], f32, tag="mean")
                            nc.vector.tensor_scalar(mean[0:64], rsum[0:64],
                                                    1.0 / N, None, OP.mult)
                            nc.vector.tensor_scalar(kTz[h][rows], kTz[h][rows],
                                                    mean[0:64], None,
                                                    OP.subtract)

            # ---- attention + deferred normalize + interleaved proj ----
            # Normalization is deferred: during the head loop only raw U
            # and the denominator row are evacuated, keeping PE dense.
            # Chunks run nh-MAJOR so the nh=0 half of normalize+proj+DMA
            # overlaps the nh=1 attention chunks.
            with tc.tile_pool(name="exp_pool", bufs=4) as epool, \
                 tc.tile_pool(name="nrm_pool", bufs=1) as npool, \
                 tc.tile_pool(name="fin", bufs=4) as fpool, \
                 tc.tile_pool(name="att_ps", bufs=1, space="PSUM") as aps:
                den_all = npool.tile([1, 2, HPC, 1024], f32, name="den_all")
                denr = npool.tile([1, 2, HPC, 1024], fr, name="denr")
                # HAM warm-up: half-array matmuls (K=64 scores / M=65 attnv)
                # never un-throttle the PE clock from cold; a short burst of
                # full-array matmuls brings it to 2.4 GHz before the head loop.
                wps = aps.tile([128, 2, 512], f32, tag="ps", bufs=3)
                for i in range(10):
                    nc.tensor.matmul(wps[:, i % 2], qT[0][:, 0:128],
                                     qT[0][:, 0:512], start=True, stop=True)
                # nh-major chunk order; flat sequence with a lag-2 pipeline
                # ACROSS chunk boundaries so ACT/DVE never drain at
                # transitions. U stays single-buffered.
                chunks = [(h, nh) for nh in range(2) for h in range(HPC)]
                seq = [(ci, mt) for ci in range(len(chunks)) for mt in range(NT)]
                Us = {}
                exs = {}

                def emit_attnv(ci, mt):
                    h, nh = chunks[ci]
                    exv = exs.pop((ci, mt))
                    for j in range(2):
                        nc.tensor.matmul(Us[ci][:, j * 512:(j + 1) * 512],
                                         v5[:, mt * HPC + h, :],
                                         exv[:, j * 512:(j + 1) * 512],
                                         start=(mt == 0), stop=(mt == NT - 1))
                    if mt == NT - 1:
                        p = h // 2
                        off = 64 * (h % 2)
                        nc.vector.tensor_copy(
                            outT[p][off:off + 64, nh * 1024:(nh + 1) * 1024],
                            Us[ci][0:64, :])
                        nc.vector.tensor_copy(den_all[:, nh, h], Us[ci][64:65, :])
                        del Us[ci]

                def emit_recip(nh, hs=slice(0, HPC)):
                    # reciprocal of denominators: DMA-reshape to 128
                    # partitions so the iterative divide runs on few
                    # elems/lane instead of thousands on one lane.
                    nhp = (hs.stop - hs.start) * 8
                    den128 = npool.tile([128, 32], f32, tag="den128", bufs=3)
                    nc.sync.dma_start(
                        den128[:, 0:nhp],
                        den_all[:, nh, hs].rearrange("o h f -> o (h f)"))
                    der128 = npool.tile([128, 32], fr, tag="der128", bufs=3)
                    with nc.allow_low_precision("softmax recip"):
                        nc.vector.reciprocal(der128[:, 0:nhp], den128[:, 0:nhp])
                    nc.gpsimd.dma_start(
                        denr[:, nh, hs].rearrange("o h f -> o (h f)"),
                        der128[:, 0:nhp])

                def emit_recip_act(nh, h):
                    # 1/den = exp(-ln(den)) on ACT: skips the DMA reshape
                    # round-trip; used for the last head where the DMA
                    # latency would stall the projection tail
                    lnb = npool.tile([1, 1024], f32, tag="lnb", bufs=2)
                    nc.scalar.activation(lnb[:], den_all[:, nh, h], ACTF.Ln)
                    nc.scalar.activation(denr[:, nh, h], lnb[:], ACTF.Exp,
                                         scale=-1.0)

                def emit_norm(nh, h):
                    # broadcast 1/den across the 64 head-dim rows via a
                    # K=1 ones matmul, then normalize outT on DVE
                    p = h // 2
                    off = 64 * (h % 2)
                    rt = aps.tile([128, 2, 512], f32, tag="ps", bufs=3,
                                  name=f"rt{nh}{h}")
                    rbp = rt[0:64].rearrange("p a b -> p (a b)")
                    for j in range(2):
                        nc.tensor.matmul(rbp[:, j * 512:(j + 1) * 512],
                                         ones_t[:],
                                         denr[:, nh, h, j * 512:(j + 1) * 512],
                                         start=True, stop=True)
                    sl = outT[p][off:off + 64, nh * 1024:(nh + 1) * 1024]
                    nc.vector.tensor_mul(sl, sl, rbp[:])

                def emit_proj(trange):
                    for t in trange:
                        ts_ = slice(t * 128, (t + 1) * 128)
                        po = aps.tile([128, 2, 512], f32, tag="ps", bufs=3)
                        for p in range(2):
                            for j in range(2):
                                nc.tensor.matmul(po[:, j],
                                                 outT[p][:, ts_],
                                                 wp_s[:, p, j * 512:(j + 1) * 512],
                                                 start=(p == 0), stop=(p == 1))
                        fin = fpool.tile([128, 1024], f32, tag="fin")
                        if t % 2 == 0:
                            nc.vector.tensor_copy(
                                fin[:], po[:].rearrange("p a b -> p (a b)"))
                        else:
                            nc.scalar.copy(
                                fin[:], po[:].rearrange("p a b -> p (a b)"))
                        eng = (nc.sync, nc.gpsimd, nc.scalar)[t % 3]
                        eng.dma_start(out[ts_, :], fin[:])

                inserts = {
                    (4, 3): lambda: emit_recip(0),
                    (5, 1): lambda: emit_norm(0, 0),
                    (5, 6): lambda: emit_norm(0, 1),
                    (5, 11): lambda: emit_norm(0, 2),
                    (6, 0): lambda: emit_norm(0, 3),
                    (6, 5): lambda: emit_proj(range(0, 2)),
                    (6, 10): lambda: emit_proj(range(2, 4)),
                    (6, 15): lambda: emit_recip(1, slice(0, 2)),
                    (7, 2): lambda: emit_recip(1, slice(2, 3)),
                    (7, 5): lambda: emit_norm(1, 0),
                    (7, 8): lambda: emit_norm(1, 1),
                    (7, 11): lambda: emit_norm(1, 2),
                    (7, 14): lambda: emit_proj(range(4, 8)),
                }

                for i, (ci, mt) in enumerate(seq):
                    h, nh = chunks[ci]
                    if mt == 0:
                        Us[ci] = aps.tile([128, 1024], f32, tag="U", bufs=1,
                                          name=f"U{ci}")
                    p = h // 2
                    off = 64 * (h % 2)
                    ms = slice(mt * 128, (mt + 1) * 128)
                    ps = aps.tile([128, 2, 512], f32, tag="ps", bufs=3)
                    for j in range(2):
                        ns = slice(nh * 1024 + j * 512,
                                   nh * 1024 + (j + 1) * 512)
                        nc.tensor.matmul(ps[:, j], kTz[h][:, ms],
                                         qT[p][:, ns],
                                         start=True, stop=True)
                    ex = epool.tile([128, 1024], bf, tag="ex", bufs=4)
                    psv = ps[:].rearrange("p a b -> p (a b)")
                    if ln_trivial:
                        if mt in DVE_MTS:
                            nc.vector.tensor_scalar(
                                ex[:].bitcast(i16), psv,
                                rs_sch[:, mt, h:h + 1], SCH_B,
                                OP.mult, OP.add)
                        else:
                            nc.scalar.activation(ex[:], psv, ACTF.Exp,
                                                 scale=rs_k[:, mt, h:h + 1])
                    else:
                        if mt in DVE_MTS:
                            nc.vector.tensor_scalar(
                                ex[:].bitcast(i16), psv,
                                SCALE * SCH_A, SCH_B, OP.mult, OP.add)
                        else:
                            nc.scalar.activation(ex[:], psv, ACTF.Exp,
                                                 scale=SCALE)
                    exs[(ci, mt)] = ex
                    if i >= 2:
                        emit_attnv(*seq[i - 2])
                    cb = inserts.pop((ci, mt), None)
                    if cb is not None:
                        cb()
                for i in (len(seq) - 2, len(seq) - 1):
                    emit_attnv(*seq[i])

                # ---- nh=1 tail: recip/normalize/proj + output DMA -----
                emit_recip_act(1, 3)
                emit_norm(1, 3)
                emit_proj(range(8, 16))

    return nc


def _get_program(ln_trivial: bool):
    key = ln_trivial
    if key not in _nc_cache:
        _install_patch()
        _nc_cache[key] = _build_program(ln_trivial)
    return _nc_cache[key]


def _bf16():
    import ml_dtypes
    return ml_dtypes.bfloat16


def _prep_core_inputs(c, x, qkv_w, q_norm_w, q_norm_b, k_norm_w, k_norm_b,
                      proj_w, ln_trivial):
    b = c // 4
    g = c % 4
    rows = slice(g * DPC, (g + 1) * DPC)
    b16 = _bf16()
    xt = np.ascontiguousarray(x[b].T).astype(b16)           # [C, N]
    wk = qkv_w[C:2 * C, :][rows, :].T                        # [C, DPC]
    wq = qkv_w[rows, :].T
    wv = qkv_w[2 * C:3 * C, :][rows, :].T
    wqkv = np.ascontiguousarray(
        np.concatenate([wk, wq, wv], axis=1)).astype(b16)    # [C, 3*DPC]
    wp = np.ascontiguousarray(proj_w[:, rows].T).astype(b16)  # [DPC, C]
    m = {"xt": xt, "wqkv": wqkv, "wp": wp,
         "ident": np.eye(128, dtype=_bf16()),
         "ones64": np.ones((1, D), dtype=np.float32)}
    if not ln_trivial:
        for nm, arr in (("gqb", q_norm_w), ("bqb", q_norm_b),
                        ("gkb", k_norm_w), ("bkb", k_norm_b)):
            t = np.broadcast_to(arr.astype(np.float32), (128, 4, D))
            m[nm] = np.ascontiguousarray(t)
    return m


def kernel(x, qkv_w, q_norm_w, q_norm_b, k_norm_w, k_norm_b, proj_w, proj_b,
           _trace=False):
    from concourse.bass_utils import run_bass_kernel_spmd

    x = np.asarray(x, dtype=np.float32)
    qkv_w = np.asarray(qkv_w, dtype=np.float32)
    q_norm_w = np.asarray(q_norm_w, dtype=np.float32)
    q_norm_b = np.asarray(q_norm_b, dtype=np.float32)
    k_norm_w = np.asarray(k_norm_w, dtype=np.float32)
    k_norm_b = np.asarray(k_norm_b, dtype=np.float32)
    proj_w = np.asarray(proj_w, dtype=np.float32)
    proj_b = np.asarray(proj_b, dtype=np.float32)

    ln_trivial = (np.all(q_norm_w == 1.0) and np.all(q_norm_b == 0.0)
                  and np.all(k_norm_w == 1.0) and np.all(k_norm_b == 0.0))

    nc = _get_program(ln_trivial)
    in_maps = [
        _prep_core_inputs(c, x, qkv_w, q_norm_w, q_norm_b, k_norm_w,
                          k_norm_b, proj_w, ln_trivial)
        for c in range(NCORES)
    ]
    res = run_bass_kernel_spmd(nc, in_maps, list(range(NCORES)),
                               trace=_trace)
    outs = [res.results[c]["out"] for c in range(NCORES)]
    full = np.empty((B, N, C), dtype=np.float32)
    for b in range(B):
        acc = outs[4 * b].astype(np.float32)
        for g in range(1, 4):
            acc = acc + outs[4 * b + g]
        full[b] = acc + proj_b[None, :]
    if _trace:
        return full, res
    return full
